# revision 13
# baseline (speedup 1.0000x reference)
"""Trainium2 Bass kernel for the BVAE sampling problem.

Contract: kernel(**inputs) takes the FULL (unsharded) numpy inputs and
returns the FULL outputs (same structure as the reference: a 7-tuple).
Internally shards tokens (T*B) across 8 NeuronCores, replicating the MLP
weights and the (7, 10000) basis_mcmc table on every core.

Design notes:
  * All matmuls run as float32r (single-pass fp32 on the PE, 1 cycle/row
    at free-dim >= 256) accumulating in fp32 PSUM.
  * Encoder runs weight-stationary ("Form F", activations features-major);
    x is transposed on-chip via PE-transpose.  Layer 3 and the decoder
    output layers run "Form T" (lhsT = activations) so results land
    token-major and DMA out contiguously.
  * The per-(t,b,z,i) gather basis_mcmc[i, idx] runs on the GPSIMD engine
    via ap_gather: each 16-partition group holds one (replicated) basis
    row, indices are host-prewrapped into the engine's int16 layout.  The
    gathers depend only on mcmc_idx, so they overlap the whole kernel.
  * pdf_approx = sum_j ind_j * f(x_j) with f a piecewise cubic:
    pdf = sum_{k,m} c_km * R_km,  R_km = sum_j ind_j * u_k(x_j) * x_j^m,
    c = A @ weights (A = block-diag piecewise-polynomial matrix, applied
    with a small PE matmul on the transposed weights).
  * ELU is computed exactly as min(exp(x+b)-1, x+b); sigmoid as
    1/(1+exp(-x)) so the only ACT table set used is Exp (no table swaps).
"""

import os
import sys
from contextlib import ExitStack

import numpy as np

sys.path.insert(0, "/opt/trn_rl_repo")

import concourse.bass as bass  # noqa: E402
import concourse.tile as tile  # noqa: E402
from concourse import bacc, mybir  # noqa: E402
from concourse.bass_utils import run_bass_kernel_spmd  # noqa: E402
from concourse.masks import make_identity  # noqa: E402

P = 128
NCORES = 8
T_MC, BATCH, D_IN, Z_DIM, NB = 32, 512, 784, 40, 7
N_MCMC = 10000
TEMP = 0.1
TOK = T_MC * BATCH          # 16384 tokens
TPC = TOK // NCORES         # 2048 tokens per core
ET = 256                    # encoder/decoder token tile
NT = TPC // ET              # 8 tiles per core
CH = 128                    # latent chunk (tokens)
NCH = TPC // CH             # 16 chunks per core
HALF = 64                   # gather granularity (tokens per ap_gather call)
NIDX = HALF * Z_DIM         # 2560 indices per gather call
D_PAD = 896                 # 784 padded to 7*128
KX = D_PAD // P             # 7 k-tiles for encoder L1

F32 = mybir.dt.float32
F32R = mybir.dt.float32r
I16 = mybir.dt.int16
AF = mybir.ActivationFunctionType
OP = mybir.AluOpType

KNOTS = np.array([0.0] * 4 + [0.25, 0.5, 0.75] + [1.0] * 4, dtype=np.float64)
DEG = 3
BASIS_INTEGRAL = ((KNOTS[DEG + 1:] - KNOTS[:NB]) / (DEG + 1)).astype(np.float64)


def _bspline_all_np(x, t, k):
    """Cox-de Boor, numpy port of the reference (float64)."""
    m = t.shape[0]
    xe = x[..., None]
    N = ((xe >= t[:-1]) & (xe < t[1:])).astype(np.float64)
    for d in range(1, k + 1):
        tl, tld = t[: m - 1 - d], t[d: m - 1]
        tr, trd = t[1: m - d], t[d + 1: m]
        den_l = tld - tl
        den_r = trd - tr
        left = np.where(den_l > 0, (xe - tl) / np.where(den_l > 0, den_l, 1.0), 0.0)
        left = left * N[..., : m - 1 - d]
        right = np.where(den_r > 0, (trd - xe) / np.where(den_r > 0, den_r, 1.0), 0.0)
        right = right * N[..., 1: m - d]
        N = left + right
    return N


def _spline_poly_matrix():
    """A_mat[i, 4*k+m]: coefficient of x^m of B_i on interval [k/4,(k+1)/4)."""
    A = np.zeros((NB, 16), dtype=np.float64)
    for k in range(4):
        lo = 0.25 * k
        pts = lo + 0.25 * np.array([0.13, 0.41, 0.63, 0.87])
        V = np.vander(pts, 4, increasing=True)       # (4 pts, 4 powers)
        B = _bspline_all_np(pts, KNOTS, DEG)         # (4 pts, 7 basis)
        C = np.linalg.solve(V, B)                    # (4 powers, 7 basis)
        for m in range(4):
            A[:, 4 * k + m] = C[m, :]
    return A


def _host_constants():
    A_mat = _spline_poly_matrix()
    # block-diagonal (280, 640): rows (z,i), cols (z, k*4+m)
    A_full = np.zeros((Z_DIM * NB, Z_DIM * 16), dtype=np.float32)
    for z in range(Z_DIM):
        A_full[z * NB:(z + 1) * NB, z * 16:(z + 1) * 16] = A_mat
    A_padded = np.zeros((3 * P, Z_DIM * 16), dtype=np.float32)
    A_padded[: Z_DIM * NB] = A_full
    A_pack = A_padded.reshape(3, P, Z_DIM * 16).transpose(1, 0, 2).copy()  # (128,3,640)

    inv_i = (1.0 / BASIS_INTEGRAL).astype(np.float32)
    invI = np.broadcast_to(inv_i, (P, NB)).copy()                 # (128, 7)
    C4 = np.broadcast_to(
        np.array([0.25, 0.5, 0.75, 1.0], np.float32), (P, 4)
    ).copy()                                                      # (128, 4)
    return A_pack, invI, C4


def _pack_w(w, kt):
    """(K, N) -> (128, kt, N) with zero padding of K to kt*128."""
    K, N = w.shape
    out = np.zeros((kt * P, N), dtype=np.float32)
    out[:K] = w
    return out.reshape(kt, P, N).transpose(1, 0, 2).copy()


def _pack_b(b):
    """(M,) -> (128, M//128) per-partition bias slices."""
    M = b.shape[0]
    return b.reshape(M // P, P).T.copy()


def _wrap_idx(idx_core):
    """(2048, 40, 7) int -> (32, 128, 160) int16 wrapped ap_gather layout.

    Call ch = chunk*2 + half covers tokens [ch*64, (ch+1)*64).
    Within a call, slot i = 40*tok_local + z; the slot-i index lives at
    partition 16*j + i%16, column i//16.
    """
    a = idx_core.reshape(NCH * 2, HALF, Z_DIM, NB)        # (32, 64, 40, 7)
    a = a.transpose(0, 3, 1, 2).reshape(NCH * 2, NB, NIDX)  # (32, 7, 2560)
    a = a.reshape(NCH * 2, NB, NIDX // 16, 16).transpose(0, 1, 3, 2)  # (32,7,16,160)
    out = np.zeros((NCH * 2, P, NIDX // 16), dtype=np.int16)
    out[:, : NB * 16] = a.reshape(NCH * 2, NB * 16, NIDX // 16)
    return out


# ---------------------------------------------------------------------------
# Bass program
# ---------------------------------------------------------------------------

_NC = None


def _mm(nc, out, lhsT, rhs, start, stop):
    nc.tensor.matmul(out, lhsT.bitcast(F32R), rhs.bitcast(F32R), start=start, stop=stop)


def _elu(nc, pool, psum_ap, bias_ap, out_ap, width):
    """out = elu(psum + bias) = exp(min(xb,0)) - 1 + max(xb,0); bias per-partition."""
    m = pool.tile([P, width], F32, tag=f"elu_m{width}")
    nc.vector.tensor_scalar(m[:], psum_ap, bias_ap, 0.0, op0=OP.add, op1=OP.min)
    r = pool.tile([P, width], F32, tag=f"elu_r{width}")
    nc.vector.tensor_scalar(r[:], psum_ap, bias_ap, 0.0, op0=OP.add, op1=OP.max)
    e = pool.tile([P, width], F32, tag=f"elu_e{width}")
    nc.scalar.activation(e[:], m[:], AF.Exp)
    nc.vector.scalar_tensor_tensor(out_ap, e[:], -1.0, r[:], op0=OP.add, op1=OP.add)


def _build_nc():
    nc = bacc.Bacc("TRN2", target_bir_lowering=False, debug=False)

    # ---- DRAM I/O (per-core shard shapes) ----
    d = {}
    d["x"] = nc.dram_tensor("x", [TPC, D_PAD], F32, kind="ExternalInput")
    d["gum"] = nc.dram_tensor("gum", [TPC, Z_DIM * NB], F32, kind="ExternalInput")
    d["idxw"] = nc.dram_tensor("idxw", [NCH * 2, P, NIDX // 16], I16, kind="ExternalInput")
    d["tbl"] = nc.dram_tensor("tbl", [NB, N_MCMC], F32, kind="ExternalInput")
    d["W1p"] = nc.dram_tensor("W1p", [P, KX, 1024], F32, kind="ExternalInput")
    d["W2p"] = nc.dram_tensor("W2p", [P, 8, 512], F32, kind="ExternalInput")
    d["We2p"] = nc.dram_tensor("We2p", [P, 4, 360], F32, kind="ExternalInput")
    d["Wd0p"] = nc.dram_tensor("Wd0p", [P, 512], F32, kind="ExternalInput")
    d["Wd1p"] = nc.dram_tensor("Wd1p", [P, 4, 1024], F32, kind="ExternalInput")
    d["Wd2p"] = nc.dram_tensor("Wd2p", [P, 8, 784], F32, kind="ExternalInput")
    d["Wv0p"] = nc.dram_tensor("Wv0p", [P, 512], F32, kind="ExternalInput")
    d["Wv1p"] = nc.dram_tensor("Wv1p", [P, 4, 1024], F32, kind="ExternalInput")
    d["Wv2p"] = nc.dram_tensor("Wv2p", [P, 8, 784], F32, kind="ExternalInput")
    d["be0p"] = nc.dram_tensor("be0p", [P, 8], F32, kind="ExternalInput")
    d["be1p"] = nc.dram_tensor("be1p", [P, 4], F32, kind="ExternalInput")
    d["bd0p"] = nc.dram_tensor("bd0p", [P, 4], F32, kind="ExternalInput")
    d["bd1p"] = nc.dram_tensor("bd1p", [P, 8], F32, kind="ExternalInput")
    d["bv0p"] = nc.dram_tensor("bv0p", [P, 4], F32, kind="ExternalInput")
    d["bv1p"] = nc.dram_tensor("bv1p", [P, 8], F32, kind="ExternalInput")
    d["be2r"] = nc.dram_tensor("be2r", [P, 360], F32, kind="ExternalInput")
    d["bd2r"] = nc.dram_tensor("bd2r", [P, 784], F32, kind="ExternalInput")
    d["bv2r"] = nc.dram_tensor("bv2r", [P, 784], F32, kind="ExternalInput")
    d["Apack"] = nc.dram_tensor("Apack", [P, 3, Z_DIM * 16], F32, kind="ExternalInput")
    d["invI"] = nc.dram_tensor("invI", [P, NB], F32, kind="ExternalInput")
    d["C4"] = nc.dram_tensor("C4", [P, 4], F32, kind="ExternalInput")

    o = {}
    o["recon_mean"] = nc.dram_tensor("recon_mean", [TPC, 784], F32, kind="ExternalOutput")
    o["recon_var"] = nc.dram_tensor("recon_var", [TPC, 784], F32, kind="ExternalOutput")
    o["coef"] = nc.dram_tensor("coef", [TPC, Z_DIM * NB], F32, kind="ExternalOutput")
    o["wout"] = nc.dram_tensor("wout", [TPC, Z_DIM * NB], F32, kind="ExternalOutput")
    o["z"] = nc.dram_tensor("z", [TPC, Z_DIM], F32, kind="ExternalOutput")
    o["pdf"] = nc.dram_tensor("pdf", [TPC, Z_DIM], F32, kind="ExternalOutput")
    o["zstd"] = nc.dram_tensor("zstd", [TPC, Z_DIM], F32, kind="ExternalOutput")

    dbg = bool(os.environ.get("BVAE_DEBUG"))
    if dbg:
        o["dbg_xT"] = nc.dram_tensor("dbg_xT", [P, KX * ET], F32, kind="ExternalOutput")
        o["dbg_h1"] = nc.dram_tensor("dbg_h1", [P, 8 * ET], F32, kind="ExternalOutput")
        o["dbg_h2"] = nc.dram_tensor("dbg_h2", [P, 4 * ET], F32, kind="ExternalOutput")
        o["dbg_lat"] = nc.dram_tensor("dbg_lat", [P, 360], F32, kind="ExternalOutput")
        o["dbg_xjm"] = nc.dram_tensor("dbg_xjm", [P, NB * Z_DIM], F32, kind="ExternalOutput")
        o["dbg_R"] = nc.dram_tensor("dbg_R", [P, Z_DIM * 16], F32, kind="ExternalOutput")
        o["dbg_csb"] = nc.dram_tensor("dbg_csb", [P, Z_DIM * 16], F32, kind="ExternalOutput")
        o["dbg_ind"] = nc.dram_tensor("dbg_ind", [P, Z_DIM * NB], F32, kind="ExternalOutput")

    da = {k: v.ap() for k, v in d.items()}
    oa = {k: v.ap() for k, v in o.items()}

    with tile.TileContext(nc) as tc, ExitStack() as ctx:
        gpool = ctx.enter_context(tc.tile_pool(name="glob", bufs=1))
        xjm_pool = ctx.enter_context(tc.tile_pool(name="xjm", bufs=8))
        gout_pool = ctx.enter_context(tc.tile_pool(name="gout", bufs=2))
        idxw_pool = ctx.enter_context(tc.tile_pool(name="idxwp", bufs=2))

        ident = gpool.tile([P, P], F32)
        make_identity(nc, ident[:])

        # basis table, one row per 16-partition group (row 6 also fills group 7)
        tbl = gpool.tile([P, N_MCMC], F32)
        nc.sync.dma_start(tbl[0:112, :], da["tbl"][:, None, :].to_broadcast([NB, 16, N_MCMC]))
        nc.sync.dma_start(tbl[112:128, :], da["tbl"][6:7, :].to_broadcast([16, N_MCMC]))

        lat_all = gpool.tile([P, NCH, 360], F32)
        zT = gpool.tile([P, TPC], F32R)

        # ---------------- gathers (overlap everything) ----------------
        xjm_tiles = []
        for c in range(NCH):
            xt = xjm_pool.tile([P, NB, Z_DIM], F32, tag="xjm")
            xjm_tiles.append(xt)
            for h in range(2):
                chh = c * 2 + h
                iw = idxw_pool.tile([P, NIDX // 16], I16, tag="idxw")
                nc.sync.dma_start(iw[:], da["idxw"][chh])
                go = gout_pool.tile([P, NIDX], F32, tag="gout")
                nc.gpsimd.ap_gather(
                    go[:], tbl[:], iw[:],
                    channels=P, num_elems=N_MCMC, d=1, num_idxs=NIDX,
                )
                for j in range(NB):
                    nc.sync.dma_start(
                        xt[h * HALF:(h + 1) * HALF, j, :],
                        go[16 * j:16 * j + 1, :].rearrange("p (t z) -> p t z", z=Z_DIM),
                    )

        # ---------------- phase E: encoder ----------------
        with ExitStack() as ectx:
            wp = ectx.enter_context(tc.tile_pool(name="ewts", bufs=1))
            ap_ = ectx.enter_context(tc.tile_pool(name="eact", bufs=1))
            tp = ectx.enter_context(tc.tile_pool(name="etmp", bufs=2))
            pst = ectx.enter_context(tc.tile_pool(name="pst", bufs=2, space="PSUM"))
            psm = ectx.enter_context(tc.tile_pool(name="psm", bufs=4, space="PSUM"))
            psl = ectx.enter_context(tc.tile_pool(name="psl", bufs=2, space="PSUM"))

            W1s = wp.tile([P, KX, 1024], F32R)
            nc.sync.dma_start(W1s[:], da["W1p"][:].bitcast(F32R))
            W2s = wp.tile([P, 8, 512], F32R)
            nc.sync.dma_start(W2s[:], da["W2p"][:].bitcast(F32R))
            We2s = wp.tile([P, 4, 360], F32R)
            nc.sync.dma_start(We2s[:], da["We2p"][:].bitcast(F32R))
            be0s = wp.tile([P, 8], F32)
            nc.sync.dma_start(be0s[:], da["be0p"][:])
            be1s = wp.tile([P, 4], F32)
            nc.sync.dma_start(be1s[:], da["be1p"][:])
            be2s = wp.tile([P, 360], F32)
            nc.sync.dma_start(be2s[:], da["be2r"][:])

            for t in range(NT):
                xT = ap_.tile([P, KX, ET], F32R, tag="xT")
                for cc in range(ET // P):
                    xin = tp.tile([P, D_PAD], F32, tag="xin")
                    nc.sync.dma_start(xin[:], da["x"][t * ET + cc * P: t * ET + (cc + 1) * P, :])
                    for kt in range(KX):
                        pt = pst.tile([P, P], F32, tag="ptr")
                        nc.tensor.transpose(pt[:], xin[:, kt * P:(kt + 1) * P], ident[:])
                        nc.vector.tensor_copy(xT[:, kt, cc * P:(cc + 1) * P], pt[:])

                h1 = ap_.tile([P, 8, ET], F32R, tag="h1")
                for mt in range(8):
                    pm = psm.tile([P, ET], F32, tag="pm")
                    for kt in range(KX):
                        _mm(nc, pm[:], W1s[:, kt, mt * P:(mt + 1) * P], xT[:, kt, :],
                            kt == 0, kt == KX - 1)
                    _elu(nc, tp, pm[:], be0s[:, mt: mt + 1], h1[:, mt, :], ET)

                h2 = ap_.tile([P, 4, ET], F32R, tag="h2")
                for mt in range(4):
                    pm = psm.tile([P, ET], F32, tag="pm")
                    for kt in range(8):
                        _mm(nc, pm[:], W2s[:, kt, mt * P:(mt + 1) * P], h1[:, kt, :],
                            kt == 0, kt == 7)
                    _elu(nc, tp, pm[:], be1s[:, mt: mt + 1], h2[:, mt, :], ET)

                for c2 in range(ET // P):
                    ci = t * (ET // P) + c2
                    pl = psl.tile([P, 360], F32, tag="pl")
                    for kt in range(4):
                        _mm(nc, pl[:], h2[:, kt, c2 * P:(c2 + 1) * P], We2s[:, kt, :],
                            kt == 0, kt == 3)
                    nc.vector.tensor_tensor(lat_all[:, ci, :], pl[:], be2s[:], op=OP.add)
                if dbg and t == 0:
                    nc.sync.dma_start(oa["dbg_xT"][:], xT[:].rearrange("p k t -> p (k t)").bitcast(F32))
                    nc.sync.dma_start(oa["dbg_h1"][:], h1[:].rearrange("p k t -> p (k t)"))
                    nc.sync.dma_start(oa["dbg_h2"][:], h2[:].rearrange("p k t -> p (k t)"))
                    nc.sync.dma_start(oa["dbg_lat"][:], lat_all[:, 0, :])

        # ---------------- phase L: latent / sampling ----------------
        with ExitStack() as lctx:
            lc = lctx.enter_context(tc.tile_pool(name="lconst", bufs=1))
            lp = lctx.enter_context(tc.tile_pool(name="lat", bufs=2))
            lp1 = lctx.enter_context(tc.tile_pool(name="lat1", bufs=1))
            pst = lctx.enter_context(tc.tile_pool(name="pstl", bufs=2, space="PSUM"))
            psc = lctx.enter_context(tc.tile_pool(name="pscl", bufs=2, space="PSUM"))

            Aps = lc.tile([P, 3, Z_DIM * 16], F32R)
            nc.sync.dma_start(Aps[:], da["Apack"][:].bitcast(F32R))
            invIs = lc.tile([P, NB], F32)
            nc.sync.dma_start(invIs[:], da["invI"][:])
            C4s = lc.tile([P, 4], F32)
            nc.sync.dma_start(C4s[:], da["C4"][:])

            ZN = Z_DIM * NB  # 280

            for c in range(NCH):
                lat3 = lat_all[:, c, :].rearrange("p (z i) -> p z i", i=9)
                logits = lat3[:, :, 2:9]
                sl = slice(c * CH, (c + 1) * CH)

                # coef softmax (with max-sub for robustness)
                mx = lp.tile([P, Z_DIM], F32, tag="mx")
                nc.vector.tensor_reduce(mx[:], logits, axis=mybir.AxisListType.X, op=OP.max)
                lsub = lp.tile([P, ZN], F32, tag="lsub")
                nc.vector.tensor_tensor(
                    lsub[:].rearrange("p (z j) -> p z j", j=NB), logits,
                    mx[:, :, None].to_broadcast([P, Z_DIM, NB]), op=OP.subtract)
                ec = lp.tile([P, ZN], F32, tag="ec")
                nc.scalar.activation(ec[:], lsub[:], AF.Exp)
                s = lp.tile([P, Z_DIM], F32, tag="s")
                nc.vector.tensor_reduce(
                    s[:], ec[:].rearrange("p (z j) -> p z j", j=NB),
                    axis=mybir.AxisListType.X, op=OP.add)
                r = lp.tile([P, Z_DIM], F32, tag="r")
                nc.vector.reciprocal(r[:], s[:])
                coef = lp.tile([P, ZN], F32, tag="coef")
                nc.vector.tensor_tensor(
                    coef[:].rearrange("p (z j) -> p z j", j=NB),
                    ec[:].rearrange("p (z j) -> p z j", j=NB),
                    r[:, :, None].to_broadcast([P, Z_DIM, NB]), op=OP.mult)
                nc.sync.dma_start(oa["coef"][sl], coef[:])

                wsb = lp.tile([P, 3 * P], F32, tag="wsb")  # 384 cols, pad zeroed
                nc.vector.memset(wsb[:, ZN:], 0.0)
                nc.vector.tensor_tensor(
                    wsb[:, :ZN].rearrange("p (z j) -> p z j", j=NB),
                    coef[:].rearrange("p (z j) -> p z j", j=NB),
                    invIs[:, None, :].to_broadcast([P, Z_DIM, NB]), op=OP.mult)
                nc.sync.dma_start(oa["wout"][sl], wsb[:, :ZN])

                # indicator softmax at temperature 0.1
                g = lp.tile([P, ZN], F32, tag="g")
                nc.sync.dma_start(g[:], da["gum"][sl])
                a = lp.tile([P, ZN], F32, tag="a")
                nc.vector.tensor_tensor(
                    a[:].rearrange("p (z j) -> p z j", j=NB), logits,
                    g[:].rearrange("p (z j) -> p z j", j=NB), op=OP.add)
                mx2 = lp.tile([P, Z_DIM], F32, tag="mx2")
                nc.vector.tensor_reduce(
                    mx2[:], a[:].rearrange("p (z j) -> p z j", j=NB),
                    axis=mybir.AxisListType.X, op=OP.max)
                asub = lp.tile([P, ZN], F32, tag="asub")
                nc.vector.tensor_tensor(
                    asub[:].rearrange("p (z j) -> p z j", j=NB),
                    a[:].rearrange("p (z j) -> p z j", j=NB),
                    mx2[:, :, None].to_broadcast([P, Z_DIM, NB]), op=OP.subtract)
                e2 = lp.tile([P, ZN], F32, tag="e2")
                nc.scalar.activation(e2[:], asub[:], AF.Exp, scale=1.0 / TEMP)
                s2 = lp.tile([P, Z_DIM], F32, tag="s2")
                nc.vector.tensor_reduce(
                    s2[:], e2[:].rearrange("p (z j) -> p z j", j=NB),
                    axis=mybir.AxisListType.X, op=OP.add)
                r2 = lp.tile([P, Z_DIM], F32, tag="r2")
                nc.vector.reciprocal(r2[:], s2[:])

                A5 = lp1.tile([P, 5, ZN], F32, tag="A5")
                nc.vector.tensor_tensor(
                    A5[:, 0, :].rearrange("p (z j) -> p z j", j=NB),
                    e2[:].rearrange("p (z j) -> p z j", j=NB),
                    r2[:, :, None].to_broadcast([P, Z_DIM, NB]), op=OP.mult)

                # sample-plane features
                xt = xjm_tiles[c]
                X4 = xt[:].rearrange("p j z -> p z j")[:, None, :, :].to_broadcast(
                    [P, 4, Z_DIM, NB])
                y = lp1.tile([P, 4, ZN], F32, tag="y")
                nc.vector.tensor_tensor(  # scratch: s'_k = (x >= (k+1)/4)
                    y[:].rearrange("p k (z j) -> p k z j", j=NB), X4,
                    C4s[:, :, None, None].to_broadcast([P, 4, Z_DIM, NB]), op=OP.is_ge)
                nc.vector.tensor_tensor(  # a_k = ind * s'_k
                    A5[:, 1:5, :].rearrange("p k (z j) -> p k z j", j=NB),
                    y[:].rearrange("p k (z j) -> p k z j", j=NB),
                    A5[:, 0, :].rearrange("p (z j) -> p z j", j=NB)[:, None, :, :]
                    .to_broadcast([P, 4, Z_DIM, NB]), op=OP.mult)
                nc.vector.tensor_tensor(  # y_k = a_k - a_{k+1} (a_0 = ind)
                    y[:], A5[:, 0:4, :], A5[:, 1:5, :], op=OP.subtract)
                p1 = lp1.tile([P, 4, ZN], F32, tag="p1")
                nc.vector.tensor_tensor(
                    p1[:].rearrange("p k (z j) -> p k z j", j=NB),
                    y[:].rearrange("p k (z j) -> p k z j", j=NB), X4, op=OP.mult)
                p2 = lp1.tile([P, 4, ZN], F32, tag="p2")
                nc.vector.tensor_tensor(
                    p2[:].rearrange("p k (z j) -> p k z j", j=NB),
                    p1[:].rearrange("p k (z j) -> p k z j", j=NB), X4, op=OP.mult)
                p3 = lp1.tile([P, 4, ZN], F32, tag="p3")
                nc.vector.tensor_tensor(
                    p3[:].rearrange("p k (z j) -> p k z j", j=NB),
                    p2[:].rearrange("p k (z j) -> p k z j", j=NB), X4, op=OP.mult)

                R = lp.tile([P, Z_DIM * 16], F32, tag="R")
                Rv = R[:].rearrange("p (z k m) -> p k z m", k=4, m=4)
                for m, pm_t in enumerate((y, p1, p2, p3)):
                    nc.vector.tensor_reduce(
                        Rv[:, :, :, m],
                        pm_t[:].rearrange("p k (z j) -> p k z j", j=NB),
                        axis=mybir.AxisListType.X, op=OP.add)

                # c = A @ w  via PE (wT from 3 PE-transposes of wsb)
                wts = []
                for kb in range(3):
                    ptr = pst.tile([P, P], F32, tag="ptw")
                    nc.tensor.transpose(ptr[:], wsb[:, kb * P:(kb + 1) * P], ident[:])
                    wt = lp.tile([P, P], F32R, tag=f"wt{kb}")
                    nc.vector.tensor_copy(wt[:], ptr[:])
                    wts.append(wt)
                csb = lp.tile([P, Z_DIM * 16], F32, tag="csb")
                for hf in range(2):
                    pc = psc.tile([P, 320], F32, tag="pc")
                    for kb in range(3):
                        _mm(nc, pc[:], wts[kb][:], Aps[:, kb, hf * 320:(hf + 1) * 320],
                            kb == 0, kb == 2)
                    nc.vector.tensor_copy(csb[:, hf * 320:(hf + 1) * 320], pc[:])

                if dbg and c == 0:
                    nc.sync.dma_start(oa["dbg_xjm"][:], xt[:].rearrange("p j z -> p (j z)"))
                    nc.sync.dma_start(oa["dbg_R"][:], R[:])
                    nc.sync.dma_start(oa["dbg_csb"][:], csb[:])
                    nc.sync.dma_start(oa["dbg_ind"][:], A5[:, 0, :])
                prodt = lp1.tile([P, Z_DIM * 16], F32, tag="prodt")
                nc.vector.tensor_tensor(prodt[:], csb[:], R[:], op=OP.mult)
                pdf = lp.tile([P, Z_DIM], F32, tag="pdf")
                nc.vector.tensor_reduce(
                    pdf[:], prodt[:].rearrange("p (z km) -> p z km", km=16),
                    axis=mybir.AxisListType.X, op=OP.add)
                nc.sync.dma_start(oa["pdf"][sl], pdf[:])

                # spl = sum_j ind_j * x_j  (exact, mask-free)
                sx = lp.tile([P, ZN], F32, tag="sx")
                nc.vector.tensor_tensor(
                    sx[:].rearrange("p (z j) -> p z j", j=NB),
                    A5[:, 0, :].rearrange("p (z j) -> p z j", j=NB),
                    xt[:].rearrange("p j z -> p z j"), op=OP.mult)
                spl = lp.tile([P, Z_DIM], F32, tag="spl")
                nc.vector.tensor_reduce(
                    spl[:], sx[:].rearrange("p (z j) -> p z j", j=NB),
                    axis=mybir.AxisListType.X, op=OP.add)

                zstd = lp.tile([P, Z_DIM], F32, tag="zstd")
                nc.scalar.activation(zstd[:], lat3[:, :, 1], AF.Exp, scale=0.5)
                nc.sync.dma_start(oa["zstd"][sl], zstd[:])
                zt = lp.tile([P, P], F32, tag="zt")
                nc.vector.memset(zt[:, Z_DIM:], 0.0)
                nc.vector.tensor_tensor(zt[:, :Z_DIM], spl[:], zstd[:], op=OP.mult)
                nc.vector.tensor_tensor(zt[:, :Z_DIM], zt[:, :Z_DIM], lat3[:, :, 0], op=OP.add)
                nc.sync.dma_start(oa["z"][sl], zt[:, :Z_DIM])
                ptz = pst.tile([P, P], F32, tag="ptw")
                nc.tensor.transpose(ptz[:], zt[:], ident[:])
                nc.vector.tensor_copy(zT[:, sl], ptz[:])

        # ---------------- phase D: decoder (branches sequential) ----------------
        for W0n, W1n, W2n, b0n, b1n, b2n, outn in (
            ("Wd0p", "Wd1p", "Wd2p", "bd0p", "bd1p", "bd2r", "recon_mean"),
            ("Wv0p", "Wv1p", "Wv2p", "bv0p", "bv1p", "bv2r", "recon_var"),
        ):
            with ExitStack() as dctx:
                wp = dctx.enter_context(tc.tile_pool(name="dwts", bufs=1))
                ap_ = dctx.enter_context(tc.tile_pool(name="dact", bufs=1))
                tp = dctx.enter_context(tc.tile_pool(name="dtmp", bufs=2))
                psm = dctx.enter_context(tc.tile_pool(name="psmd", bufs=4, space="PSUM"))
                psl = dctx.enter_context(tc.tile_pool(name="psld", bufs=4, space="PSUM"))

                W0s = wp.tile([P, 512], F32R)
                nc.sync.dma_start(W0s[:], da[W0n][:].bitcast(F32R))
                W1s = wp.tile([P, 4, 1024], F32R)
                nc.sync.dma_start(W1s[:], da[W1n][:].bitcast(F32R))
                W2s = wp.tile([P, 8, 784], F32R)
                nc.sync.dma_start(W2s[:], da[W2n][:].bitcast(F32R))
                b0s = wp.tile([P, 4], F32)
                nc.sync.dma_start(b0s[:], da[b0n][:])
                b1s = wp.tile([P, 8], F32)
                nc.sync.dma_start(b1s[:], da[b1n][:])
                b2s = wp.tile([P, 784], F32)
                nc.sync.dma_start(b2s[:], da[b2n][:])

                for t in range(NT):
                    tsl = slice(t * ET, (t + 1) * ET)
                    h1 = ap_.tile([P, 4, ET], F32R, tag="h1d")
                    for mt in range(4):
                        pm = psm.tile([P, ET], F32, tag="pmd")
                        _mm(nc, pm[:], W0s[:, mt * P:(mt + 1) * P], zT[:, tsl], True, True)
                        _elu(nc, tp, pm[:], b0s[:, mt: mt + 1], h1[:, mt, :], ET)
                    h2 = ap_.tile([P, 8, ET], F32R, tag="h2d")
                    for mt in range(8):
                        pm = psm.tile([P, ET], F32, tag="pmd")
                        for kt in range(4):
                            _mm(nc, pm[:], W1s[:, kt, mt * P:(mt + 1) * P], h1[:, kt, :],
                                kt == 0, kt == 3)
                        _elu(nc, tp, pm[:], b1s[:, mt: mt + 1], h2[:, mt, :], ET)

                    for c2 in range(ET // P):
                        osl = slice(t * ET + c2 * P, t * ET + (c2 + 1) * P)
                        xb = tp.tile([P, 784], F32, tag="sxb")
                        for hf in range(2):
                            pl = psl.tile([P, 392], F32, tag="pld")
                            for kt in range(8):
                                _mm(nc, pl[:], h2[:, kt, c2 * P:(c2 + 1) * P],
                                    W2s[:, kt, hf * 392:(hf + 1) * 392], kt == 0, kt == 7)
                            nc.vector.tensor_tensor(
                                xb[:, hf * 392:(hf + 1) * 392], pl[:],
                                b2s[:, hf * 392:(hf + 1) * 392], op=OP.add)
                        es = tp.tile([P, 784], F32, tag="ses")
                        nc.scalar.activation(es[:], xb[:], AF.Exp, scale=-1.0)
                        ds = tp.tile([P, 784], F32, tag="sds")
                        nc.vector.tensor_scalar(ds[:], es[:], 1.0, None, op0=OP.add)
                        og = tp.tile([P, 784], F32, tag="sog")
                        nc.vector.reciprocal(og[:], ds[:])
                        nc.sync.dma_start(oa[outn][osl], og[:])

    nc.compile()
    return nc


def _get_nc():
    global _NC
    if _NC is None:
        _NC = _build_nc()
    return _NC


def make_core_inputs(x, We0, be0, We1, be1, We2, be2, Wd0, bd0, Wd1, bd1, Wd2, bd2,
                     Wv0, bv0, Wv1, bv1, Wv2, bv2, basis_mcmc, gumbel, mcmc_idx):
    """Host-side prep: shard + pack.  Returns list of 8 in_maps."""
    A_pack, invI, C4 = _host_constants()
    f = np.float32
    shared = {
        "tbl": np.ascontiguousarray(basis_mcmc, f),
        "W1p": _pack_w(np.asarray(We0, f), KX),
        "W2p": _pack_w(np.asarray(We1, f), 8),
        "We2p": _pack_w(np.asarray(We2, f), 4),
        "Wd0p": _pack_w(np.asarray(Wd0, f), 1)[:, 0, :],
        "Wd1p": _pack_w(np.asarray(Wd1, f), 4),
        "Wd2p": _pack_w(np.asarray(Wd2, f), 8),
        "Wv0p": _pack_w(np.asarray(Wv0, f), 1)[:, 0, :],
        "Wv1p": _pack_w(np.asarray(Wv1, f), 4),
        "Wv2p": _pack_w(np.asarray(Wv2, f), 8),
        "be0p": _pack_b(np.asarray(be0, f)),
        "be1p": _pack_b(np.asarray(be1, f)),
        "bd0p": _pack_b(np.asarray(bd0, f)),
        "bd1p": _pack_b(np.asarray(bd1, f)),
        "bv0p": _pack_b(np.asarray(bv0, f)),
        "bv1p": _pack_b(np.asarray(bv1, f)),
        "be2r": np.broadcast_to(np.asarray(be2, f), (P, 360)).copy(),
        "bd2r": np.broadcast_to(np.asarray(bd2, f), (P, 784)).copy(),
        "bv2r": np.broadcast_to(np.asarray(bv2, f), (P, 784)).copy(),
        "Apack": A_pack,
        "invI": invI,
        "C4": C4,
    }
    xf = np.zeros((TOK, D_PAD), f)
    xf[:, :D_IN] = np.asarray(x, f).reshape(TOK, D_IN)
    gf = np.asarray(gumbel, f).reshape(TOK, Z_DIM * NB)
    idxf = np.asarray(mcmc_idx).reshape(TOK, Z_DIM, NB)

    in_maps = []
    for c in range(NCORES):
        sl = slice(c * TPC, (c + 1) * TPC)
        m = dict(shared)
        m["x"] = np.ascontiguousarray(xf[sl])
        m["gum"] = np.ascontiguousarray(gf[sl])
        m["idxw"] = _wrap_idx(idxf[sl])
        in_maps.append(m)
    return in_maps


def assemble_outputs(results):
    """results: list of 8 dicts -> reference-shaped 7-tuple."""
    def cat(name):
        return np.concatenate([r[name] for r in results], axis=0)

    recon_mean = cat("recon_mean").reshape(T_MC, BATCH, D_IN)
    recon_var = cat("recon_var").reshape(T_MC, BATCH, D_IN)
    coef = cat("coef").reshape(T_MC, BATCH, Z_DIM, NB)
    wout = cat("wout").reshape(T_MC, BATCH, Z_DIM, NB)
    z = cat("z").reshape(T_MC, BATCH, Z_DIM)
    pdf = cat("pdf").reshape(T_MC, BATCH, Z_DIM)
    zstd = cat("zstd").reshape(T_MC, BATCH, Z_DIM)
    return (recon_mean, recon_var, coef, wout, z, pdf, zstd)


def kernel(**inputs):
    nc = _get_nc()
    in_maps = make_core_inputs(**inputs)
    res = run_bass_kernel_spmd(nc, in_maps, core_ids=list(range(NCORES)))
    return assemble_outputs(res.results)


# revision 15
# speedup vs baseline: 998.8407x; 998.8407x over previous
"""Trainium2 Bass kernel for the BVAE sampling problem.

Contract: kernel(**inputs) takes the FULL (unsharded) numpy inputs and
returns the FULL outputs (same structure as the reference: a 7-tuple).
Internally shards tokens (T*B) across 8 NeuronCores, replicating the MLP
weights and the (7, 10000) basis_mcmc table on every core.

Design notes:
  * All matmuls run as float32r (single-pass fp32 on the PE, 1 cycle/row
    at free-dim >= 256) accumulating in fp32 PSUM.
  * Encoder runs weight-stationary ("Form F", activations features-major);
    x is transposed on-chip via PE-transpose.  Layer 3 and the decoder
    output layers run "Form T" (lhsT = activations) so results land
    token-major and DMA out contiguously.
  * The per-(t,b,z,i) gather basis_mcmc[i, idx] runs on the GPSIMD engine
    via ap_gather: each 16-partition group holds one (replicated) basis
    row, indices are host-prewrapped into the engine's int16 layout.  The
    gathers depend only on mcmc_idx, so they overlap the whole kernel.
  * pdf_approx = sum_j ind_j * f(x_j) with f a piecewise cubic:
    pdf = sum_{k,m} c_km * R_km,  R_km = sum_j ind_j * u_k(x_j) * x_j^m,
    c = A @ weights (A = block-diag piecewise-polynomial matrix, applied
    with a small PE matmul on the transposed weights).
  * ELU is computed exactly as min(exp(x+b)-1, x+b); sigmoid as
    1/(1+exp(-x)) so the only ACT table set used is Exp (no table swaps).
"""

import os
import sys
from contextlib import ExitStack

import numpy as np

sys.path.insert(0, "/opt/trn_rl_repo")

import concourse.bass as bass  # noqa: E402
import concourse.tile as tile  # noqa: E402
from concourse import bacc, mybir  # noqa: E402
from concourse.bass_utils import run_bass_kernel_spmd  # noqa: E402
from concourse.masks import make_identity  # noqa: E402

P = 128
NCORES = 8
T_MC, BATCH, D_IN, Z_DIM, NB = 32, 512, 784, 40, 7
N_MCMC = 10000
TEMP = 0.1
TOK = T_MC * BATCH          # 16384 tokens
TPC = TOK // NCORES         # 2048 tokens per core
ET = 256                    # encoder/decoder token tile
NT = TPC // ET              # 8 tiles per core
CH = 128                    # latent chunk (tokens)
NCH = TPC // CH             # 16 chunks per core
HALF = 64                   # gather granularity (tokens per ap_gather call)
NIDX = HALF * Z_DIM         # 2560 indices per gather call
D_PAD = 896                 # 784 padded to 7*128
KX = D_PAD // P             # 7 k-tiles for encoder L1

F32 = mybir.dt.float32
F32R = mybir.dt.float32r
I16 = mybir.dt.int16
AF = mybir.ActivationFunctionType
OP = mybir.AluOpType

KNOTS = np.array([0.0] * 4 + [0.25, 0.5, 0.75] + [1.0] * 4, dtype=np.float64)
DEG = 3
BASIS_INTEGRAL = ((KNOTS[DEG + 1:] - KNOTS[:NB]) / (DEG + 1)).astype(np.float64)


def _bspline_all_np(x, t, k):
    """Cox-de Boor, numpy port of the reference (float64)."""
    m = t.shape[0]
    xe = x[..., None]
    N = ((xe >= t[:-1]) & (xe < t[1:])).astype(np.float64)
    for d in range(1, k + 1):
        tl, tld = t[: m - 1 - d], t[d: m - 1]
        tr, trd = t[1: m - d], t[d + 1: m]
        den_l = tld - tl
        den_r = trd - tr
        left = np.where(den_l > 0, (xe - tl) / np.where(den_l > 0, den_l, 1.0), 0.0)
        left = left * N[..., : m - 1 - d]
        right = np.where(den_r > 0, (trd - xe) / np.where(den_r > 0, den_r, 1.0), 0.0)
        right = right * N[..., 1: m - d]
        N = left + right
    return N


def _spline_poly_matrix():
    """A_mat[i, 4*k+m]: coefficient of x^m of B_i on interval [k/4,(k+1)/4)."""
    A = np.zeros((NB, 16), dtype=np.float64)
    for k in range(4):
        lo = 0.25 * k
        pts = lo + 0.25 * np.array([0.13, 0.41, 0.63, 0.87])
        V = np.vander(pts, 4, increasing=True)       # (4 pts, 4 powers)
        B = _bspline_all_np(pts, KNOTS, DEG)         # (4 pts, 7 basis)
        C = np.linalg.solve(V, B)                    # (4 powers, 7 basis)
        for m in range(4):
            A[:, 4 * k + m] = C[m, :]
    return A


def _host_constants():
    A_mat = _spline_poly_matrix()
    # block-diagonal (280, 640): rows (z,i), cols (z, k*4+m)
    A_full = np.zeros((Z_DIM * NB, Z_DIM * 16), dtype=np.float32)
    for z in range(Z_DIM):
        A_full[z * NB:(z + 1) * NB, z * 16:(z + 1) * 16] = A_mat
    A_padded = np.zeros((3 * P, Z_DIM * 16), dtype=np.float32)
    A_padded[: Z_DIM * NB] = A_full
    A_pack = A_padded.reshape(3, P, Z_DIM * 16).transpose(1, 0, 2).copy()  # (128,3,640)

    inv_i = (1.0 / BASIS_INTEGRAL).astype(np.float32)
    invI = np.broadcast_to(inv_i, (P, NB)).copy()                 # (128, 7)
    C4 = np.broadcast_to(
        np.array([0.25, 0.5, 0.75, 1.0], np.float32), (P, 4)
    ).copy()                                                      # (128, 4)
    return A_pack, invI, C4


def _pack_w(w, kt):
    """(K, N) -> (128, kt, N) with zero padding of K to kt*128."""
    K, N = w.shape
    out = np.zeros((kt * P, N), dtype=np.float32)
    out[:K] = w
    return out.reshape(kt, P, N).transpose(1, 0, 2).copy()


def _pack_b(b):
    """(M,) -> (128, M//128) per-partition bias slices."""
    M = b.shape[0]
    return b.reshape(M // P, P).T.copy()


def _wrap_idx(idx_core):
    """(2048, 40, 7) int -> (32, 128, 160) int16 wrapped ap_gather layout.

    Call ch = chunk*2 + half covers tokens [ch*64, (ch+1)*64).
    Within a call, slot i = 40*tok_local + z; the slot-i index lives at
    partition 16*j + i%16, column i//16.
    """
    a = idx_core.reshape(NCH * 2, HALF, Z_DIM, NB)        # (32, 64, 40, 7)
    a = a.transpose(0, 3, 1, 2).reshape(NCH * 2, NB, NIDX)  # (32, 7, 2560)
    a = a.reshape(NCH * 2, NB, NIDX // 16, 16).transpose(0, 1, 3, 2)  # (32,7,16,160)
    out = np.zeros((NCH * 2, P, NIDX // 16), dtype=np.int16)
    out[:, : NB * 16] = a.reshape(NCH * 2, NB * 16, NIDX // 16)
    return out


# ---------------------------------------------------------------------------
# Bass program
# ---------------------------------------------------------------------------

_NC = None


def _mm(nc, out, lhsT, rhs, start, stop):
    nc.tensor.matmul(out, lhsT.bitcast(F32R), rhs.bitcast(F32R), start=start, stop=stop)


def _elu(nc, pool, psum_ap, bias_ap, out_ap, width):
    """out = elu(psum + bias) = exp(min(xb,0)) - 1 + max(xb,0); bias per-partition."""
    m = pool.tile([P, width], F32, tag=f"elu_m{width}")
    nc.vector.tensor_scalar(m[:], psum_ap, bias_ap, 0.0, op0=OP.add, op1=OP.min)
    r = pool.tile([P, width], F32, tag=f"elu_r{width}")
    nc.vector.tensor_scalar(r[:], psum_ap, bias_ap, 0.0, op0=OP.add, op1=OP.max)
    e = pool.tile([P, width], F32, tag=f"elu_e{width}")
    nc.scalar.activation(e[:], m[:], AF.Exp)
    nc.vector.scalar_tensor_tensor(out_ap, e[:], -1.0, r[:], op0=OP.add, op1=OP.add)


def _build_nc():
    nc = bacc.Bacc("TRN2", target_bir_lowering=False, debug=False)

    # ---- DRAM I/O (per-core shard shapes) ----
    d = {}
    d["x"] = nc.dram_tensor("x", [TPC, D_PAD], F32, kind="ExternalInput")
    d["gum"] = nc.dram_tensor("gum", [TPC, Z_DIM * NB], F32, kind="ExternalInput")
    d["idxw"] = nc.dram_tensor("idxw", [NCH * 2, P, NIDX // 16], I16, kind="ExternalInput")
    d["tbl"] = nc.dram_tensor("tbl", [NB, N_MCMC], F32, kind="ExternalInput")
    d["W1p"] = nc.dram_tensor("W1p", [P, KX, 1024], F32, kind="ExternalInput")
    d["W2p"] = nc.dram_tensor("W2p", [P, 8, 512], F32, kind="ExternalInput")
    d["We2p"] = nc.dram_tensor("We2p", [P, 4, 360], F32, kind="ExternalInput")
    d["Wd0p"] = nc.dram_tensor("Wd0p", [P, 512], F32, kind="ExternalInput")
    d["Wd1p"] = nc.dram_tensor("Wd1p", [P, 4, 1024], F32, kind="ExternalInput")
    d["Wd2p"] = nc.dram_tensor("Wd2p", [P, 8, 784], F32, kind="ExternalInput")
    d["Wv0p"] = nc.dram_tensor("Wv0p", [P, 512], F32, kind="ExternalInput")
    d["Wv1p"] = nc.dram_tensor("Wv1p", [P, 4, 1024], F32, kind="ExternalInput")
    d["Wv2p"] = nc.dram_tensor("Wv2p", [P, 8, 784], F32, kind="ExternalInput")
    d["be0p"] = nc.dram_tensor("be0p", [P, 8], F32, kind="ExternalInput")
    d["be1p"] = nc.dram_tensor("be1p", [P, 4], F32, kind="ExternalInput")
    d["bd0p"] = nc.dram_tensor("bd0p", [P, 4], F32, kind="ExternalInput")
    d["bd1p"] = nc.dram_tensor("bd1p", [P, 8], F32, kind="ExternalInput")
    d["bv0p"] = nc.dram_tensor("bv0p", [P, 4], F32, kind="ExternalInput")
    d["bv1p"] = nc.dram_tensor("bv1p", [P, 8], F32, kind="ExternalInput")
    d["be2r"] = nc.dram_tensor("be2r", [P, 360], F32, kind="ExternalInput")
    d["bd2r"] = nc.dram_tensor("bd2r", [P, 784], F32, kind="ExternalInput")
    d["bv2r"] = nc.dram_tensor("bv2r", [P, 784], F32, kind="ExternalInput")
    d["Apack"] = nc.dram_tensor("Apack", [P, 3, Z_DIM * 16], F32, kind="ExternalInput")
    d["invI"] = nc.dram_tensor("invI", [P, NB], F32, kind="ExternalInput")
    d["C4"] = nc.dram_tensor("C4", [P, 4], F32, kind="ExternalInput")

    o = {}
    o["recon_mean"] = nc.dram_tensor("recon_mean", [TPC, 784], F32, kind="ExternalOutput")
    o["recon_var"] = nc.dram_tensor("recon_var", [TPC, 784], F32, kind="ExternalOutput")
    o["coef"] = nc.dram_tensor("coef", [TPC, Z_DIM * NB], F32, kind="ExternalOutput")
    o["wout"] = nc.dram_tensor("wout", [TPC, Z_DIM * NB], F32, kind="ExternalOutput")
    o["z"] = nc.dram_tensor("z", [TPC, Z_DIM], F32, kind="ExternalOutput")
    o["pdf"] = nc.dram_tensor("pdf", [TPC, Z_DIM], F32, kind="ExternalOutput")
    o["zstd"] = nc.dram_tensor("zstd", [TPC, Z_DIM], F32, kind="ExternalOutput")

    dbg = bool(os.environ.get("BVAE_DEBUG"))
    if dbg:
        o["dbg_xT"] = nc.dram_tensor("dbg_xT", [P, KX * ET], F32, kind="ExternalOutput")
        o["dbg_h1"] = nc.dram_tensor("dbg_h1", [P, 8 * ET], F32, kind="ExternalOutput")
        o["dbg_h2"] = nc.dram_tensor("dbg_h2", [P, 4 * ET], F32, kind="ExternalOutput")
        o["dbg_lat"] = nc.dram_tensor("dbg_lat", [P, 360], F32, kind="ExternalOutput")
        o["dbg_xjm"] = nc.dram_tensor("dbg_xjm", [P, NB * Z_DIM], F32, kind="ExternalOutput")
        o["dbg_R"] = nc.dram_tensor("dbg_R", [P, Z_DIM * 16], F32, kind="ExternalOutput")
        o["dbg_csb"] = nc.dram_tensor("dbg_csb", [P, Z_DIM * 16], F32, kind="ExternalOutput")
        o["dbg_ind"] = nc.dram_tensor("dbg_ind", [P, Z_DIM * NB], F32, kind="ExternalOutput")

    da = {k: v.ap() for k, v in d.items()}
    oa = {k: v.ap() for k, v in o.items()}

    with tile.TileContext(nc) as tc, ExitStack() as ctx:
        gpool = ctx.enter_context(tc.tile_pool(name="glob", bufs=1))
        xjm_pool = ctx.enter_context(tc.tile_pool(name="xjm", bufs=8))
        gout_pool = ctx.enter_context(tc.tile_pool(name="gout", bufs=2))
        idxw_pool = ctx.enter_context(tc.tile_pool(name="idxwp", bufs=2))

        ident = gpool.tile([P, P], F32)
        make_identity(nc, ident[:])

        # basis table, one row per 16-partition group (row 6 also fills group 7)
        tbl = gpool.tile([P, N_MCMC], F32)
        nc.sync.dma_start(tbl[0:112, :], da["tbl"][:, None, :].to_broadcast([NB, 16, N_MCMC]))
        nc.sync.dma_start(tbl[112:128, :], da["tbl"][6:7, :].to_broadcast([16, N_MCMC]))

        lat_all = gpool.tile([P, NCH, 360], F32)
        zT = gpool.tile([P, TPC], F32R)

        # ---------------- gathers (overlap everything) ----------------
        xjm_tiles = []
        for c in range(NCH):
            xt = xjm_pool.tile([P, NB, Z_DIM], F32, tag="xjm")
            xjm_tiles.append(xt)
            for h in range(2):
                chh = c * 2 + h
                iw = idxw_pool.tile([P, NIDX // 16], I16, tag="idxw")
                nc.sync.dma_start(iw[:], da["idxw"][chh])
                go = gout_pool.tile([P, NIDX], F32, tag="gout")
                nc.gpsimd.ap_gather(
                    go[:], tbl[:], iw[:],
                    channels=P, num_elems=N_MCMC, d=1, num_idxs=NIDX,
                )
                for j in range(NB):
                    nc.sync.dma_start(
                        xt[h * HALF:(h + 1) * HALF, j, :],
                        go[16 * j:16 * j + 1, :].rearrange("p (t z) -> p t z", z=Z_DIM),
                    )

        # ---------------- phase E: encoder ----------------
        with ExitStack() as ectx:
            wp = ectx.enter_context(tc.tile_pool(name="ewts", bufs=1))
            ap_ = ectx.enter_context(tc.tile_pool(name="eact", bufs=1))
            tp = ectx.enter_context(tc.tile_pool(name="etmp", bufs=2))
            pst = ectx.enter_context(tc.tile_pool(name="pst", bufs=2, space="PSUM"))
            psm = ectx.enter_context(tc.tile_pool(name="psm", bufs=4, space="PSUM"))
            psl = ectx.enter_context(tc.tile_pool(name="psl", bufs=2, space="PSUM"))

            W1s = wp.tile([P, KX, 1024], F32R)
            nc.sync.dma_start(W1s[:], da["W1p"][:].bitcast(F32R))
            W2s = wp.tile([P, 8, 512], F32R)
            nc.sync.dma_start(W2s[:], da["W2p"][:].bitcast(F32R))
            We2s = wp.tile([P, 4, 360], F32R)
            nc.sync.dma_start(We2s[:], da["We2p"][:].bitcast(F32R))
            be0s = wp.tile([P, 8], F32)
            nc.sync.dma_start(be0s[:], da["be0p"][:])
            be1s = wp.tile([P, 4], F32)
            nc.sync.dma_start(be1s[:], da["be1p"][:])
            be2s = wp.tile([P, 360], F32)
            nc.sync.dma_start(be2s[:], da["be2r"][:])

            for t in range(NT):
                xT = ap_.tile([P, KX, ET], F32R, tag="xT")
                for cc in range(ET // P):
                    xin = tp.tile([P, D_PAD], F32, tag="xin")
                    nc.sync.dma_start(xin[:], da["x"][t * ET + cc * P: t * ET + (cc + 1) * P, :])
                    for kt in range(KX):
                        pt = pst.tile([P, P], F32, tag="ptr")
                        nc.tensor.transpose(pt[:], xin[:, kt * P:(kt + 1) * P], ident[:])
                        nc.vector.tensor_copy(xT[:, kt, cc * P:(cc + 1) * P], pt[:])

                h1 = ap_.tile([P, 8, ET], F32R, tag="h1")
                for mt in range(8):
                    pm = psm.tile([P, ET], F32, tag="pm")
                    for kt in range(KX):
                        _mm(nc, pm[:], W1s[:, kt, mt * P:(mt + 1) * P], xT[:, kt, :],
                            kt == 0, kt == KX - 1)
                    _elu(nc, tp, pm[:], be0s[:, mt: mt + 1], h1[:, mt, :], ET)

                h2 = ap_.tile([P, 4, ET], F32R, tag="h2")
                for mt in range(4):
                    pm = psm.tile([P, ET], F32, tag="pm")
                    for kt in range(8):
                        _mm(nc, pm[:], W2s[:, kt, mt * P:(mt + 1) * P], h1[:, kt, :],
                            kt == 0, kt == 7)
                    _elu(nc, tp, pm[:], be1s[:, mt: mt + 1], h2[:, mt, :], ET)

                for c2 in range(ET // P):
                    ci = t * (ET // P) + c2
                    pl = psl.tile([P, 360], F32, tag="pl")
                    for kt in range(4):
                        _mm(nc, pl[:], h2[:, kt, c2 * P:(c2 + 1) * P], We2s[:, kt, :],
                            kt == 0, kt == 3)
                    nc.vector.tensor_tensor(lat_all[:, ci, :], pl[:], be2s[:], op=OP.add)
                if dbg and t == 0:
                    nc.sync.dma_start(oa["dbg_xT"][:], xT[:].rearrange("p k t -> p (k t)").bitcast(F32))
                    nc.sync.dma_start(oa["dbg_h1"][:], h1[:].rearrange("p k t -> p (k t)"))
                    nc.sync.dma_start(oa["dbg_h2"][:], h2[:].rearrange("p k t -> p (k t)"))
                    nc.sync.dma_start(oa["dbg_lat"][:], lat_all[:, 0, :])

        # ---------------- phase L: latent / sampling ----------------
        with ExitStack() as lctx:
            lc = lctx.enter_context(tc.tile_pool(name="lconst", bufs=1))
            lp = lctx.enter_context(tc.tile_pool(name="lat", bufs=2))
            lp1 = lctx.enter_context(tc.tile_pool(name="lat1", bufs=1))
            pst = lctx.enter_context(tc.tile_pool(name="pstl", bufs=2, space="PSUM"))
            psc = lctx.enter_context(tc.tile_pool(name="pscl", bufs=2, space="PSUM"))

            Aps = lc.tile([P, 3, Z_DIM * 16], F32R)
            nc.sync.dma_start(Aps[:], da["Apack"][:].bitcast(F32R))
            invIs = lc.tile([P, NB], F32)
            nc.sync.dma_start(invIs[:], da["invI"][:])
            C4s = lc.tile([P, 4], F32)
            nc.sync.dma_start(C4s[:], da["C4"][:])

            ZN = Z_DIM * NB  # 280

            for c in range(NCH):
                lat3 = lat_all[:, c, :].rearrange("p (z i) -> p z i", i=9)
                logits = lat3[:, :, 2:9]
                sl = slice(c * CH, (c + 1) * CH)

                # coef softmax (with max-sub for robustness)
                mx = lp.tile([P, Z_DIM], F32, tag="mx")
                nc.vector.tensor_reduce(mx[:], logits, axis=mybir.AxisListType.X, op=OP.max)
                lsub = lp.tile([P, ZN], F32, tag="lsub")
                nc.vector.tensor_tensor(
                    lsub[:].rearrange("p (z j) -> p z j", j=NB), logits,
                    mx[:, :, None].to_broadcast([P, Z_DIM, NB]), op=OP.subtract)
                ec = lp.tile([P, ZN], F32, tag="ec")
                nc.scalar.activation(ec[:], lsub[:], AF.Exp)
                s = lp.tile([P, Z_DIM], F32, tag="s")
                nc.vector.tensor_reduce(
                    s[:], ec[:].rearrange("p (z j) -> p z j", j=NB),
                    axis=mybir.AxisListType.X, op=OP.add)
                r = lp.tile([P, Z_DIM], F32, tag="r")
                nc.vector.reciprocal(r[:], s[:])
                coef = lp.tile([P, ZN], F32, tag="coef")
                nc.vector.tensor_tensor(
                    coef[:].rearrange("p (z j) -> p z j", j=NB),
                    ec[:].rearrange("p (z j) -> p z j", j=NB),
                    r[:, :, None].to_broadcast([P, Z_DIM, NB]), op=OP.mult)
                nc.sync.dma_start(oa["coef"][sl], coef[:])

                wsb = lp.tile([P, 3 * P], F32, tag="wsb")  # 384 cols, pad zeroed
                nc.vector.memset(wsb[:, ZN:], 0.0)
                nc.vector.tensor_tensor(
                    wsb[:, :ZN].rearrange("p (z j) -> p z j", j=NB),
                    coef[:].rearrange("p (z j) -> p z j", j=NB),
                    invIs[:, None, :].to_broadcast([P, Z_DIM, NB]), op=OP.mult)
                nc.sync.dma_start(oa["wout"][sl], wsb[:, :ZN])

                # indicator softmax at temperature 0.1
                g = lp.tile([P, ZN], F32, tag="g")
                nc.sync.dma_start(g[:], da["gum"][sl])
                a = lp.tile([P, ZN], F32, tag="a")
                nc.vector.tensor_tensor(
                    a[:].rearrange("p (z j) -> p z j", j=NB), logits,
                    g[:].rearrange("p (z j) -> p z j", j=NB), op=OP.add)
                mx2 = lp.tile([P, Z_DIM], F32, tag="mx2")
                nc.vector.tensor_reduce(
                    mx2[:], a[:].rearrange("p (z j) -> p z j", j=NB),
                    axis=mybir.AxisListType.X, op=OP.max)
                asub = lp.tile([P, ZN], F32, tag="asub")
                nc.vector.tensor_tensor(
                    asub[:].rearrange("p (z j) -> p z j", j=NB),
                    a[:].rearrange("p (z j) -> p z j", j=NB),
                    mx2[:, :, None].to_broadcast([P, Z_DIM, NB]), op=OP.subtract)
                e2 = lp.tile([P, ZN], F32, tag="e2")
                nc.scalar.activation(e2[:], asub[:], AF.Exp, scale=1.0 / TEMP)
                s2 = lp.tile([P, Z_DIM], F32, tag="s2")
                nc.vector.tensor_reduce(
                    s2[:], e2[:].rearrange("p (z j) -> p z j", j=NB),
                    axis=mybir.AxisListType.X, op=OP.add)
                r2 = lp.tile([P, Z_DIM], F32, tag="r2")
                nc.vector.reciprocal(r2[:], s2[:])

                A5 = lp1.tile([P, 5, ZN], F32, tag="A5")
                nc.vector.tensor_tensor(
                    A5[:, 0, :].rearrange("p (z j) -> p z j", j=NB),
                    e2[:].rearrange("p (z j) -> p z j", j=NB),
                    r2[:, :, None].to_broadcast([P, Z_DIM, NB]), op=OP.mult)

                # sample-plane features
                xt = xjm_tiles[c]
                X4 = xt[:].rearrange("p j z -> p z j")[:, None, :, :].to_broadcast(
                    [P, 4, Z_DIM, NB])
                y = lp1.tile([P, 4, ZN], F32, tag="y")
                nc.vector.tensor_tensor(  # scratch: s'_k = (x >= (k+1)/4)
                    y[:].rearrange("p k (z j) -> p k z j", j=NB), X4,
                    C4s[:, :, None, None].to_broadcast([P, 4, Z_DIM, NB]), op=OP.is_ge)
                nc.vector.tensor_tensor(  # a_k = ind * s'_k
                    A5[:, 1:5, :].rearrange("p k (z j) -> p k z j", j=NB),
                    y[:].rearrange("p k (z j) -> p k z j", j=NB),
                    A5[:, 0, :].rearrange("p (z j) -> p z j", j=NB)[:, None, :, :]
                    .to_broadcast([P, 4, Z_DIM, NB]), op=OP.mult)
                nc.vector.tensor_tensor(  # y_k = a_k - a_{k+1} (a_0 = ind)
                    y[:], A5[:, 0:4, :], A5[:, 1:5, :], op=OP.subtract)
                p1 = lp1.tile([P, 4, ZN], F32, tag="p1")
                nc.vector.tensor_tensor(
                    p1[:].rearrange("p k (z j) -> p k z j", j=NB),
                    y[:].rearrange("p k (z j) -> p k z j", j=NB), X4, op=OP.mult)
                p2 = lp1.tile([P, 4, ZN], F32, tag="p2")
                nc.vector.tensor_tensor(
                    p2[:].rearrange("p k (z j) -> p k z j", j=NB),
                    p1[:].rearrange("p k (z j) -> p k z j", j=NB), X4, op=OP.mult)
                p3 = lp1.tile([P, 4, ZN], F32, tag="p3")
                nc.vector.tensor_tensor(
                    p3[:].rearrange("p k (z j) -> p k z j", j=NB),
                    p2[:].rearrange("p k (z j) -> p k z j", j=NB), X4, op=OP.mult)

                R = lp.tile([P, Z_DIM * 16], F32, tag="R")
                Rv = R[:].rearrange("p (z k m) -> p k z m", k=4, m=4)
                for m, pm_t in enumerate((y, p1, p2, p3)):
                    nc.vector.tensor_reduce(
                        Rv[:, :, :, m],
                        pm_t[:].rearrange("p k (z j) -> p k z j", j=NB),
                        axis=mybir.AxisListType.X, op=OP.add)

                # c = A @ w  via PE (wT from 3 PE-transposes of wsb)
                wts = []
                for kb in range(3):
                    ptr = pst.tile([P, P], F32, tag="ptw")
                    nc.tensor.transpose(ptr[:], wsb[:, kb * P:(kb + 1) * P], ident[:])
                    wt = lp.tile([P, P], F32R, tag=f"wt{kb}")
                    nc.vector.tensor_copy(wt[:], ptr[:])
                    wts.append(wt)
                csb = lp.tile([P, Z_DIM * 16], F32, tag="csb")
                for hf in range(2):
                    pc = psc.tile([P, 320], F32, tag="pc")
                    for kb in range(3):
                        _mm(nc, pc[:], wts[kb][:], Aps[:, kb, hf * 320:(hf + 1) * 320],
                            kb == 0, kb == 2)
                    nc.vector.tensor_copy(csb[:, hf * 320:(hf + 1) * 320], pc[:])

                if dbg and c == 0:
                    nc.sync.dma_start(oa["dbg_xjm"][:], xt[:].rearrange("p j z -> p (j z)"))
                    nc.sync.dma_start(oa["dbg_R"][:], R[:])
                    nc.sync.dma_start(oa["dbg_csb"][:], csb[:])
                    nc.sync.dma_start(oa["dbg_ind"][:], A5[:, 0, :])
                prodt = lp1.tile([P, Z_DIM * 16], F32, tag="prodt")
                nc.vector.tensor_tensor(prodt[:], csb[:], R[:], op=OP.mult)
                pdf = lp.tile([P, Z_DIM], F32, tag="pdf")
                nc.vector.tensor_reduce(
                    pdf[:], prodt[:].rearrange("p (z km) -> p z km", km=16),
                    axis=mybir.AxisListType.X, op=OP.add)
                nc.sync.dma_start(oa["pdf"][sl], pdf[:])

                # spl = sum_j ind_j * x_j  (exact, mask-free)
                sx = lp.tile([P, ZN], F32, tag="sx")
                nc.vector.tensor_tensor(
                    sx[:].rearrange("p (z j) -> p z j", j=NB),
                    A5[:, 0, :].rearrange("p (z j) -> p z j", j=NB),
                    xt[:].rearrange("p j z -> p z j"), op=OP.mult)
                spl = lp.tile([P, Z_DIM], F32, tag="spl")
                nc.vector.tensor_reduce(
                    spl[:], sx[:].rearrange("p (z j) -> p z j", j=NB),
                    axis=mybir.AxisListType.X, op=OP.add)

                zstd = lp.tile([P, Z_DIM], F32, tag="zstd")
                nc.scalar.activation(zstd[:], lat3[:, :, 1], AF.Exp, scale=0.5)
                nc.sync.dma_start(oa["zstd"][sl], zstd[:])
                zt = lp.tile([P, P], F32, tag="zt")
                nc.vector.memset(zt[:, Z_DIM:], 0.0)
                nc.vector.tensor_tensor(zt[:, :Z_DIM], spl[:], zstd[:], op=OP.mult)
                nc.vector.tensor_tensor(zt[:, :Z_DIM], zt[:, :Z_DIM], lat3[:, :, 0], op=OP.add)
                nc.sync.dma_start(oa["z"][sl], zt[:, :Z_DIM])
                ptz = pst.tile([P, P], F32, tag="ptw")
                nc.tensor.transpose(ptz[:], zt[:], ident[:])
                nc.vector.tensor_copy(zT[:, sl], ptz[:])

        # ---------------- phase D: decoder (branches sequential) ----------------
        for W0n, W1n, W2n, b0n, b1n, b2n, outn in (
            ("Wd0p", "Wd1p", "Wd2p", "bd0p", "bd1p", "bd2r", "recon_mean"),
            ("Wv0p", "Wv1p", "Wv2p", "bv0p", "bv1p", "bv2r", "recon_var"),
        ):
            with ExitStack() as dctx:
                wp = dctx.enter_context(tc.tile_pool(name="dwts", bufs=1))
                ap_ = dctx.enter_context(tc.tile_pool(name="dact", bufs=1))
                tp = dctx.enter_context(tc.tile_pool(name="dtmp", bufs=2))
                psm = dctx.enter_context(tc.tile_pool(name="psmd", bufs=4, space="PSUM"))
                psl = dctx.enter_context(tc.tile_pool(name="psld", bufs=4, space="PSUM"))

                W0s = wp.tile([P, 512], F32R)
                nc.sync.dma_start(W0s[:], da[W0n][:].bitcast(F32R))
                W1s = wp.tile([P, 4, 1024], F32R)
                nc.sync.dma_start(W1s[:], da[W1n][:].bitcast(F32R))
                W2s = wp.tile([P, 8, 784], F32R)
                nc.sync.dma_start(W2s[:], da[W2n][:].bitcast(F32R))
                b0s = wp.tile([P, 4], F32)
                nc.sync.dma_start(b0s[:], da[b0n][:])
                b1s = wp.tile([P, 8], F32)
                nc.sync.dma_start(b1s[:], da[b1n][:])
                b2s = wp.tile([P, 784], F32)
                nc.sync.dma_start(b2s[:], da[b2n][:])

                for t in range(NT):
                    tsl = slice(t * ET, (t + 1) * ET)
                    h1 = ap_.tile([P, 4, ET], F32R, tag="h1d")
                    for mt in range(4):
                        pm = psm.tile([P, ET], F32, tag="pmd")
                        _mm(nc, pm[:], W0s[:, mt * P:(mt + 1) * P], zT[:, tsl], True, True)
                        _elu(nc, tp, pm[:], b0s[:, mt: mt + 1], h1[:, mt, :], ET)
                    h2 = ap_.tile([P, 8, ET], F32R, tag="h2d")
                    for mt in range(8):
                        pm = psm.tile([P, ET], F32, tag="pmd")
                        for kt in range(4):
                            _mm(nc, pm[:], W1s[:, kt, mt * P:(mt + 1) * P], h1[:, kt, :],
                                kt == 0, kt == 3)
                        _elu(nc, tp, pm[:], b1s[:, mt: mt + 1], h2[:, mt, :], ET)

                    for c2 in range(ET // P):
                        osl = slice(t * ET + c2 * P, t * ET + (c2 + 1) * P)
                        xb = tp.tile([P, 784], F32, tag="sxb")
                        for hf in range(2):
                            pl = psl.tile([P, 392], F32, tag="pld")
                            for kt in range(8):
                                _mm(nc, pl[:], h2[:, kt, c2 * P:(c2 + 1) * P],
                                    W2s[:, kt, hf * 392:(hf + 1) * 392], kt == 0, kt == 7)
                            nc.vector.tensor_tensor(
                                xb[:, hf * 392:(hf + 1) * 392], pl[:],
                                b2s[:, hf * 392:(hf + 1) * 392], op=OP.add)
                        es = tp.tile([P, 784], F32, tag="ses")
                        nc.scalar.activation(es[:], xb[:], AF.Exp, scale=-1.0)
                        ds = tp.tile([P, 784], F32, tag="sds")
                        nc.vector.tensor_scalar(ds[:], es[:], 1.0, None, op0=OP.add)
                        og = tp.tile([P, 784], F32, tag="sog")
                        nc.vector.reciprocal(og[:], ds[:])
                        nc.sync.dma_start(oa[outn][osl], og[:])

    nc.compile()
    return nc


def _get_nc():
    global _NC
    if _NC is None:
        _NC = _build_nc()
    return _NC


def make_core_inputs(x, We0, be0, We1, be1, We2, be2, Wd0, bd0, Wd1, bd1, Wd2, bd2,
                     Wv0, bv0, Wv1, bv1, Wv2, bv2, basis_mcmc, gumbel, mcmc_idx):
    """Host-side prep: shard + pack.  Returns list of 8 in_maps."""
    A_pack, invI, C4 = _host_constants()
    f = np.float32
    shared = {
        "tbl": np.ascontiguousarray(basis_mcmc, f),
        "W1p": _pack_w(np.asarray(We0, f), KX),
        "W2p": _pack_w(np.asarray(We1, f), 8),
        "We2p": _pack_w(np.asarray(We2, f), 4),
        "Wd0p": _pack_w(np.asarray(Wd0, f), 1)[:, 0, :],
        "Wd1p": _pack_w(np.asarray(Wd1, f), 4),
        "Wd2p": _pack_w(np.asarray(Wd2, f), 8),
        "Wv0p": _pack_w(np.asarray(Wv0, f), 1)[:, 0, :],
        "Wv1p": _pack_w(np.asarray(Wv1, f), 4),
        "Wv2p": _pack_w(np.asarray(Wv2, f), 8),
        "be0p": _pack_b(np.asarray(be0, f)),
        "be1p": _pack_b(np.asarray(be1, f)),
        "bd0p": _pack_b(np.asarray(bd0, f)),
        "bd1p": _pack_b(np.asarray(bd1, f)),
        "bv0p": _pack_b(np.asarray(bv0, f)),
        "bv1p": _pack_b(np.asarray(bv1, f)),
        "be2r": np.broadcast_to(np.asarray(be2, f), (P, 360)).copy(),
        "bd2r": np.broadcast_to(np.asarray(bd2, f), (P, 784)).copy(),
        "bv2r": np.broadcast_to(np.asarray(bv2, f), (P, 784)).copy(),
        "Apack": A_pack,
        "invI": invI,
        "C4": C4,
    }
    xf = np.zeros((TOK, D_PAD), f)
    xf[:, :D_IN] = np.asarray(x, f).reshape(TOK, D_IN)
    gf = np.asarray(gumbel, f).reshape(TOK, Z_DIM * NB)
    idxf = np.asarray(mcmc_idx).reshape(TOK, Z_DIM, NB)

    in_maps = []
    for c in range(NCORES):
        sl = slice(c * TPC, (c + 1) * TPC)
        m = dict(shared)
        m["x"] = np.ascontiguousarray(xf[sl])
        m["gum"] = np.ascontiguousarray(gf[sl])
        m["idxw"] = _wrap_idx(idxf[sl])
        in_maps.append(m)
    return in_maps


def assemble_outputs(results):
    """results: list of 8 dicts -> reference-shaped 7-tuple."""
    def cat(name):
        return np.concatenate([r[name] for r in results], axis=0)

    recon_mean = cat("recon_mean").reshape(T_MC, BATCH, D_IN)
    recon_var = cat("recon_var").reshape(T_MC, BATCH, D_IN)
    coef = cat("coef").reshape(T_MC, BATCH, Z_DIM, NB)
    wout = cat("wout").reshape(T_MC, BATCH, Z_DIM, NB)
    z = cat("z").reshape(T_MC, BATCH, Z_DIM)
    pdf = cat("pdf").reshape(T_MC, BATCH, Z_DIM)
    zstd = cat("zstd").reshape(T_MC, BATCH, Z_DIM)
    return (recon_mean, recon_var, coef, wout, z, pdf, zstd)


_RUNNER = None


def _get_runner():
    """Cached jitted shard_map executor (mirrors bass2jax.run_bass_via_pjrt)."""
    global _RUNNER
    if _RUNNER is not None:
        return _RUNNER
    import jax
    from jax.sharding import Mesh, PartitionSpec, NamedSharding
    from jax.experimental.shard_map import shard_map
    from concourse import bass2jax, mybir as _mb

    bass2jax.install_neuronx_cc_hook()
    nc = _get_nc()
    part_name = nc.partition_id_tensor.name if nc.partition_id_tensor else None
    in_names, out_names, out_avals = [], [], []
    for alloc in nc.m.functions[0].allocations:
        if not isinstance(alloc, _mb.MemoryLocationSet):
            continue
        name = alloc.memorylocations[0].name
        if alloc.kind == "ExternalInput":
            if name != part_name:
                in_names.append(name)
        elif alloc.kind == "ExternalOutput":
            out_names.append(name)
            out_avals.append(jax.core.ShapedArray(
                tuple(alloc.tensor_shape), _mb.dt.np(alloc.dtype)))
    n_params = len(in_names)
    all_names = in_names + out_names
    if part_name is not None:
        all_names = all_names + [part_name]

    def _body(*args):
        operands = list(args)
        if part_name is not None:
            operands.append(bass2jax.partition_id_tensor())
        outs = bass2jax._bass_exec_p.bind(
            *operands,
            out_avals=tuple(out_avals),
            in_names=tuple(all_names),
            out_names=tuple(out_names),
            lowering_input_output_aliases=(),
            sim_require_finite=True,
            sim_require_nnan=True,
            nc=nc,
        )
        return tuple(outs)

    devices = jax.devices()[:NCORES]
    mesh = Mesh(np.asarray(devices), ("core",))
    spec = NamedSharding(mesh, PartitionSpec("core"))
    n_outs = len(out_names)
    sharded = jax.jit(shard_map(
        _body, mesh=mesh,
        in_specs=(PartitionSpec("core"),) * (n_params + n_outs),
        out_specs=(PartitionSpec("core"),) * n_outs,
        check_rep=False,
    ), keep_unused=True)
    zeros = [np.zeros((NCORES * a.shape[0], *a.shape[1:]), a.dtype) for a in out_avals]
    _RUNNER = dict(nc=nc, in_names=in_names, out_names=out_names,
                   out_avals=out_avals, sharded=sharded, mesh=mesh, spec=spec,
                   zeros=zeros)
    return _RUNNER


def run_in_maps(in_maps):
    """Execute prepared per-core in_maps; returns list of per-core dicts."""
    r = _get_runner()
    concat = [np.concatenate([np.asarray(in_maps[c][n]) for c in range(NCORES)], axis=0)
              for n in r["in_names"]]
    outs = r["sharded"](*concat, *r["zeros"])
    res = []
    for c in range(NCORES):
        m = {}
        for i, n in enumerate(r["out_names"]):
            a = np.asarray(outs[i])
            m[n] = a.reshape(NCORES, *r["out_avals"][i].shape)[c]
        res.append(m)
    return res


def kernel(**inputs):
    in_maps = make_core_inputs(**inputs)
    return assemble_outputs(run_in_maps(in_maps))
